# revision 17
# baseline (speedup 1.0000x reference)
"""Distance-weighted embedding loss on 8 Trainium2 NeuronCores.

reference:
    gathered = embedding[indices]                      # [B, K, D]
    sq = sum((gathered - emb_batch[:,None,:])**2, -1)  # [B, K]
    loss = sum(sq * attr_sim) / B                      # scalar

Sharding: data-parallel over the batch. Each of the 8 cores handles
B/8 = 512 samples; the embedding table is replicated (shipped bf16 so a
gathered row costs 256B of HBM read). Each core reduces its shard to a
single partial sum on-device; the host adds the 8 partials and divides
by B (the scalar all-reduce).

Per-core device program (Tile framework), v2 — TensorE reduction:
  - samples processed in 4 groups of 128 (partition dim = sample)
  - per group, neighbor rows are gathered 25 (last group: 10) at a time
    via indirect DMA
  - dif = gathered - x  (DVE 2x packed bf16 subtract; or, with USE_CCE,
    the gather DMA itself accumulates onto a -x prefill)
  - sq = dif**2, column-split between the scalar engine (ACT square)
    and the DVE (self-mult tensor_tensor) to balance the two engines
  - the weighted reduction over samples happens on the almost-idle
    TensorE: for each (group, k) one skinny accumulating matmul
        psum[1, D] += attr_col[128, 1].T @ sq_k[128, D]
    i.e. psum[d] = sum_{p,k} attr[p,k] * dif[p,k,d]^2.  A final
    tensor_reduce over d yields the scalar.
  - a warm-up block of dummy matmuls runs during the DMA lead-in so the
    PE HAM clock gate is already released when real matmuls arrive.
"""

import ml_dtypes
import numpy as np

import concourse.bass as bass
import concourse.tile as tile
from concourse import bacc, mybir
from concourse.bass_utils import run_bass_kernel_spmd

F32 = mybir.dt.float32
BF16 = mybir.dt.bfloat16
I32 = mybir.dt.int32

NCORES = 8
D = 128
P = 128
USE_CCE = False


def build_program(V: int, S_C: int, K: int, use_cce: bool = USE_CCE):
    """Build the per-core Bass program.

    V: table rows; S_C: samples per core (multiple of 128);
    K: neighbors per sample.
    """
    G = S_C // P
    assert S_C % P == 0

    nc = bacc.Bacc("TRN2", target_bir_lowering=False, debug=False)

    xg_d = nc.dram_tensor("xg", [P, G * D], BF16, kind="ExternalInput")
    attr_d = nc.dram_tensor("attr", [P, G * K], BF16, kind="ExternalInput")
    offs_d = nc.dram_tensor("offsets", [P, G * K], I32, kind="ExternalInput")
    table = nc.dram_tensor("embedding", [V, D], BF16, kind="ExternalInput")
    loss = nc.dram_tensor("loss", [1, 1], F32, kind="ExternalOutput")

    # chunk schedule: (k0, nct, act_cols) per group.  act_cols is the ACT
    # engine's share of the square; DVE squares the rest.  The split is
    # tuned so ACT-busy ~= DVE-busy ~= 19.5us, just under the ~20us gather
    # stream.  Small leading chunks start the compute pipeline early; the
    # tiny final chunk (with a DVE-heavy split) keeps the post-stream
    # chain short.
    first_segs = [(0, 12, 9), (12, 13, 10), (25, 25, 19)]
    wide = [(0, 25, 19), (25, 25, 19)]
    last_segs = [(0, 10, 8), (10, 10, 8), (20, 10, 8), (30, 10, 8), (40, 10, 4)]
    n_mm = G * K

    with tile.TileContext(nc) as tc:
        with (
            tc.tile_pool(name="const", bufs=1) as const,
            tc.tile_pool(name="gather", bufs=8) as gpool,
            tc.tile_pool(name="dif", bufs=6) as dpool,
            tc.tile_pool(name="sq", bufs=6) as spool,
            tc.tile_pool(name="psum", bufs=1, space="PSUM") as psum,
        ):
            # PE warm-up: ~6us of dummy matmuls during the DMA lead-in lift
            # the HAM clock gate to 2.4 GHz before the real matmuls arrive.
            wones = const.tile([P, 512], BF16)
            nc.vector.memset(wones[:], 1.0)
            wps = psum.tile([1, 512], F32)
            for _ in range(16):
                nc.tensor.matmul(
                    out=wps[:], lhsT=wones[:, :1], rhs=wones[:],
                    start=True, stop=True,
                )
            # ACT warm-up: trigger the one-time ~2.7us Square spline-table
            # load now, during the DMA lead-in, not at the first real square
            wsq = const.tile([1, 2], BF16)
            nc.scalar.square(out=wsq[:], in_=wones[:1, :2])

            # one offsets load: a split load's second half finishes ~2us
            # later (completion-sem receipt) and stalls the second gather
            offs_sb = const.tile([P, G * K], I32)
            nc.sync.dma_start(out=offs_sb[:], in_=offs_d[:])
            xg = const.tile([P, G * D], BF16)
            nc.scalar.dma_start(out=xg[:], in_=xg_d[:])
            attr_sb = const.tile([P, G * K], BF16)
            nc.scalar.dma_start(out=attr_sb[:], in_=attr_d[:])

            acc = psum.tile([1, D], F32)
            chunks = []
            for g in range(G):
                if g == 0:
                    segs = first_segs
                elif g == G - 1:
                    segs = last_segs
                else:
                    segs = wide
                for k0, nct, ca in segs:
                    chunks.append((g, k0, nct, ca))

            def emit_deferred(pend, mm_i):
                """DVE square share + the chunk's matmuls.  Deferred until
                after the NEXT chunk's subtract is emitted so the DVE FIFO
                prioritises subtracts and ACT never waits behind a
                DVE-square."""
                sq, dif, g, k0, nct, ca = pend
                if ca < nct:
                    nc.vector.tensor_tensor(
                        out=sq[:, ca * D:nct * D],
                        in0=dif[:, ca * D:nct * D],
                        in1=dif[:, ca * D:nct * D],
                        op=mybir.AluOpType.mult,
                    )
                for k in range(nct):
                    col = g * K + k0 + k
                    nc.tensor.matmul(
                        out=acc[:],
                        lhsT=attr_sb[:, col:col + 1],
                        rhs=sq[:, k * D:(k + 1) * D],
                        start=(mm_i == 0), stop=(mm_i == n_mm - 1),
                    )
                    mm_i += 1
                return mm_i

            mm_i = 0
            pend = None
            for g, k0, nct, ca in chunks:
                xg_g = xg[:, g * D:(g + 1) * D]
                if True:
                    xg_b = xg_g.unsqueeze(1).to_broadcast([P, nct, D])
                    ioff = bass.IndirectOffsetOnAxis(
                        ap=offs_sb[:, g * K + k0: g * K + k0 + nct],
                        axis=0,
                    )
                    if use_cce:
                        # prefill with -x (xg is shipped pre-negated), then
                        # the gather DMA accumulates the rows in flight:
                        # dif = -x + gathered
                        dt = dpool.tile([P, 25 * D], BF16, tag="dt")
                        dif = dt[:, :nct * D]
                        nc.vector.tensor_copy(
                            out=dif.rearrange("p (n d) -> p n d", n=nct),
                            in_=xg_b,
                        )
                        nc.gpsimd.indirect_dma_start(
                            out=dif, out_offset=None,
                            in_=table[:], in_offset=ioff,
                            compute_op=mybir.AluOpType.add,
                        )
                    else:
                        m = gpool.tile([P, 25 * D], BF16, tag="m")
                        mm = m[:, :nct * D]
                        nc.gpsimd.indirect_dma_start(
                            out=mm, out_offset=None,
                            in_=table[:], in_offset=ioff,
                        )
                        dt = dpool.tile([P, 25 * D], BF16, tag="dt")
                        dif = dt[:, :nct * D]
                        nc.vector.tensor_tensor(
                            out=dif.rearrange("p (n d) -> p n d", n=nct),
                            in0=mm.rearrange("p (n d) -> p n d", n=nct),
                            in1=xg_b,
                            op=mybir.AluOpType.subtract,
                        )

                    # ACT's share of the square goes out immediately; the
                    # DVE share + matmuls of the PREVIOUS chunk follow the
                    # subtract in the DVE FIFO (see emit_deferred)
                    ca = min(ca, nct)
                    sq = spool.tile([P, 25 * D], BF16, tag="sq")
                    nc.scalar.square(out=sq[:, :ca * D], in_=dif[:, :ca * D])
                    if pend is not None:
                        mm_i = emit_deferred(pend, mm_i)
                    pend = (sq, dif, g, k0, nct, ca)

            mm_i = emit_deferred(pend, mm_i)
            assert mm_i == n_mm

            tot = const.tile([1, 1], F32)
            nc.vector.tensor_reduce(
                out=tot[:], in_=acc[:],
                axis=mybir.AxisListType.X,
                op=mybir.AluOpType.add,
            )
            nc.sync.dma_start(out=loss[:], in_=tot[:])

    nc.compile()
    return nc


def shard_inputs(emb_batch, embedding, attr_sim, indices,
                 use_cce: bool = USE_CCE, ncores: int = NCORES):
    """Build the per-core input maps (layout/dtype prep only)."""
    B, K = attr_sim.shape
    s_c = B // ncores
    g = s_c // P
    xg_all = np.asarray(emb_batch, dtype=np.float32)
    if use_cce:
        xg_all = -xg_all
    xg_all = xg_all.astype(ml_dtypes.bfloat16)
    attr_bf = np.asarray(attr_sim, dtype=np.float32).astype(ml_dtypes.bfloat16)
    emb_bf = np.asarray(embedding, dtype=np.float32).astype(ml_dtypes.bfloat16)
    idx = np.asarray(indices).astype(np.int32)

    in_maps = []
    for c in range(ncores):
        sl = slice(c * s_c, (c + 1) * s_c)
        # [s_c, X] -> [P, G*X]: t[p, g*X + x] = src[g*128 + p, x]
        xg = np.ascontiguousarray(
            xg_all[sl].reshape(g, P, D).transpose(1, 0, 2).reshape(P, g * D))
        at = np.ascontiguousarray(
            attr_bf[sl].reshape(g, P, K).transpose(1, 0, 2).reshape(P, g * K))
        offs = np.ascontiguousarray(
            idx[sl].reshape(g, P, K).transpose(1, 0, 2).reshape(P, g * K))
        in_maps.append({
            "xg": xg,
            "attr": at,
            "offsets": offs,
            "embedding": emb_bf,
        })
    return in_maps


_cached = {}


def kernel(emb_batch, embedding, attr_sim, indices, beta):
    emb_batch = np.asarray(emb_batch)
    embedding = np.asarray(embedding)
    attr_sim = np.asarray(attr_sim)
    indices = np.asarray(indices)
    B, K = attr_sim.shape
    V = embedding.shape[0]
    key = (V, B // NCORES, K, USE_CCE)
    if key not in _cached:
        _cached[key] = build_program(V, B // NCORES, K, USE_CCE)
    nc = _cached[key]
    in_maps = shard_inputs(emb_batch, embedding, attr_sim, indices, USE_CCE)
    res = run_bass_kernel_spmd(nc, in_maps, list(range(NCORES)))
    partials = [res.results[c]["loss"][0, 0] for c in range(NCORES)]
    return np.float32(np.sum(np.asarray(partials, dtype=np.float64)) / B)
